# revision 34
# baseline (speedup 1.0000x reference)
"""Trainium2 Bass kernel for nn_DeepDendriticEncoder.

Computes, for every sliding window n of length 256 over x[0:500000]:
    h1 = relu(X @ W1.T); h2 = relu(h1 @ W2.T); h3 = relu(h2 @ W3.T)
    I[n] = 2 * max_k h3[n, k]
on 8 NeuronCores (window axis sharded, W-1 halo on x), then finishes the
tiny LIF latency / argmin chain on host in f32.

Device strategy per core (Hankel windows never materialized in DRAM):
  - per super-iteration of 4 blocks x 512 windows, one DMA brings a
    "diagonal" tile D[i, c] = x[base + i + c] (128 x 2176, overlapping
    strided read straight from HBM; big rows amortize descriptor cost)
  - conv-as-matmul: h1 = W1a.T @ D[:, b:b+512] + W1b.T @ D[:, b+128:b+640]
    accumulated in PSUM (contraction = tap index, 2 x 128); the weight
    loads amortize over the 4 blocks of a super-iteration
  - layer 2 packs two blocks per PSUM tile via column tiling
    (tile_position=(0,0)/(0,64)) so the two matmuls run concurrently on
    array column halves and relu processes 128 full partitions
  - layer 3 swaps operands (relu(h2) chunks stationary, W3.T moving) so
    h3 lands [window, k3]; block pairs run concurrently on array row
    halves (tile_position=(0,0)/(64,0)); the max over k3 is then a
    free-axis DVE reduce - no cross-partition reduction anywhere
  - per-core I values accumulate in SBUF, one DMA out at the end.

Matmul inputs run in bf16 (fp32 PSUM accumulation). The downstream
consumers are cliff functions with enormous margins for this problem
family (spike threshold I>1, integer step counts, argmax gaps), so
bf16-level error (~1e-2 relative) is far below every decision margin;
the reported winner/latency values are recomputed on host in f32.
"""

import sys

for _p in ("/opt/trn_rl_repo",):
    if _p not in sys.path:
        sys.path.insert(0, _p)

import numpy as np

# ---- problem constants (match reference.py; hardcoded by contract) ----
T = 500000
W_WIN = 256
K = 128
DT = 0.01
TAU = 0.05
DECAY = 1.0 - DT / TAU  # 0.8
MAX_STEPS = 200000
N = T - W_WIN + 1  # 499745

NCORES = 8
NPC = (N + NCORES - 1) // NCORES  # 62469 windows per core (last core fewer)
BLK = 512
SUPER = 2  # blocks per super-iteration
NSUP = (NPC + SUPER * BLK - 1) // (SUPER * BLK)  # 62
NBLK = NSUP * SUPER  # 124
CAP = NBLK * BLK  # 63488 windows computed per core (incl. padding)
XSH = CAP + 2 * K  # 63744 x-shard length
DSPAN = 4  # super-iterations per diag DMA

_compiled = None


def _build():
    """Build + compile the SPMD Bass program once per process."""
    import concourse.bass as bass
    import concourse.tile as tile
    from concourse import bacc, mybir

    f32 = mybir.dt.float32
    # fp8 e4m3 data path, fp32 PSUM accumulation. Layer 1 runs in
    # DoubleRow perf mode (2 fp8 MACs/cell/cycle, contraction 256 in one
    # matmul); e4m3's +-448 range covers every operand comfortably.
    dt1 = mybir.dt.float8e4
    dt2 = mybir.dt.float8e4
    RELU = mybir.ActivationFunctionType.Relu
    nc = bacc.Bacc("TRN2", target_bir_lowering=False)

    xs = nc.dram_tensor("xs", [XSH], dt1, kind="ExternalInput")
    w1t = nc.dram_tensor("w1t", [128, 256], dt1, kind="ExternalInput")
    w2t = nc.dram_tensor("w2t", [128, 64], dt2, kind="ExternalInput")
    # W3.T duplicated on partition halves so row-tiled layer-3 matmuls can
    # read it from partitions 0-63 and 64-127
    w3t = nc.dram_tensor("w3t", [128, 32], dt2, kind="ExternalInput")
    iout = nc.dram_tensor("iout", [128, NBLK * 4], f32, kind="ExternalOutput")

    DW = SUPER * BLK + 128  # 2176: diag tile width per super-iteration

    with tile.TileContext(nc) as tc:
        with (
            tc.tile_pool(name="const", bufs=1) as cpool,
            tc.tile_pool(name="diag", bufs=2) as dpool,
            tc.tile_pool(name="acts", bufs=2) as rpool,
            tc.tile_pool(name="iacc", bufs=1) as ipool,
            tc.tile_pool(name="psA", bufs=2, space="PSUM") as psA,
            tc.tile_pool(name="psB", bufs=2, space="PSUM") as psB,
            tc.tile_pool(name="psC", bufs=1, space="PSUM") as psC,
        ):
            w1s = cpool.tile([128, 256], dt1)
            nc.sync.dma_start(w1s[:], w1t[:])
            w2s = cpool.tile([128, 64], dt2)
            nc.sync.dma_start(w2s[:], w2t[:])
            w3s = cpool.tile([128, 32], dt2)
            nc.sync.dma_start(w3s[:], w3t[:])
            isb = ipool.tile([128, NBLK * 4], f32)

            SW = SUPER * BLK  # 1024 windows per super-iteration
            # DoubleRow weights AP: contraction = (partition i, ktile q)
            # over taps 128 q + i; free dims [q, elem] with q-step 128
            w1dr = bass.AP(
                w1s[:].tensor, w1s[:].offset,
                [list(w1s[:].ap[0]), [128, 2], [1, 128]],
            )
            for ds in range(0, NSUP, DSPAN):
                span = min(DSPAN, NSUP - ds)
                dw = span * SW + 128
                d = dpool.tile([128, DSPAN * SW + 128], dt1, tag="d")
                nc.sync.dma_start(
                    d[:, :dw], bass.AP(xs, ds * SW, [[1, 128], [1, dw]])
                )
                for s in range(ds, ds + span):
                    off = (s - ds) * SW
                    # layer 1: one DoubleRow matmul per 512-window block
                    p1 = [
                        psA.tile(
                            [128, BLK], f32, name=f"p1_{k}", tag=f"p1_{k}",
                        )
                        for k in range(SUPER)
                    ]
                    for k in range(SUPER):
                        dsl = d[:, off + BLK * k : off + BLK * k + BLK + 128]
                        ddr = bass.AP(
                            dsl.tensor, dsl.offset,
                            [list(dsl.ap[0]), [128, 2], [1, BLK]],
                        )
                        nc.tensor.matmul(
                            p1[k][:], w1dr, ddr,
                            start=True, stop=True,
                            perf_mode=mybir.MatmulPerfMode.DoubleRow,
                        )
                    # relu split across ACT and DVE to balance engine load
                    r1 = []
                    for k in range(SUPER):
                        t = rpool.tile(
                            [128, BLK], dt2, name=f"r1_{k}", tag=f"r1_{k}"
                        )
                        if k == 0:
                            nc.scalar.activation(t[:], p1[k][:], RELU)
                        else:
                            nc.vector.tensor_relu(t[:], p1[k][:])
                        r1.append(t)

                    # layer 2: the block pair packed on array column halves
                    p2 = psB.tile([128, BLK], f32, name="p2", tag="p2")
                    nc.tensor.matmul(
                        p2[0:64, :], w2s[:], r1[0][:],
                        start=True, stop=True, tile_position=(0, 0),
                    )
                    nc.tensor.matmul(
                        p2[64:128, :], w2s[:], r1[1][:],
                        start=True, stop=True, tile_position=(0, 64),
                    )
                    r2 = rpool.tile([128, BLK], dt2, name="r2", tag="r2")
                    # mostly ACT; a fifth of the supers go to DVE so the
                    # two engines end up evenly loaded
                    if s % 5 == 0:
                        nc.vector.tensor_relu(r2[:], p2[:])
                    else:
                        nc.scalar.activation(r2[:], p2[:], RELU)

                    # layer 3: stationary = relu(h2) chunks, moving = W3.T;
                    # the two blocks run concurrently on array row halves,
                    # each row group draining into its own PSUM bank
                    p3a = psC.tile([128, 128], f32, name="p3a", tag="p3a")
                    p3b = psC.tile([128, 128], f32, name="p3b", tag="p3b")
                    for c in range(4):
                        nc.tensor.matmul(
                            p3a[:, 32 * c : 32 * c + 32],
                            r2[0:64, 128 * c : 128 * (c + 1)],
                            w3s[0:64, :],
                            start=True, stop=True, tile_position=(0, 0),
                        )
                        nc.tensor.matmul(
                            p3b[:, 32 * c : 32 * c + 32],
                            r2[64:128, 128 * c : 128 * (c + 1)],
                            w3s[64:128, :],
                            start=True, stop=True, tile_position=(64, 0),
                        )
                    # p3a col 32c+k3 <-> window 1024 s + 128 c + p
                    # p3b col 32c+k3 <-> window 1024 s + 512 + 128 c + p
                    nc.vector.tensor_reduce(
                        isb[:, 8 * s : 8 * s + 4],
                        p3a[:].rearrange("p (g k) -> p g k", k=32),
                        axis=mybir.AxisListType.X,
                        op=mybir.AluOpType.max,
                    )
                    nc.vector.tensor_reduce(
                        isb[:, 8 * s + 4 : 8 * s + 8],
                        p3b[:].rearrange("p (g k) -> p g k", k=32),
                        axis=mybir.AxisListType.X,
                        op=mybir.AluOpType.max,
                    )

            nc.sync.dma_start(iout[:], isb[:])

    nc.compile()
    return nc


def _get_compiled():
    global _compiled
    if _compiled is None:
        _compiled = _build()
    return _compiled


def _run_device(x, W1, W2, W3, trace=False):
    """Shard across 8 cores, run, return full pre-activation max array [N]."""
    import ml_dtypes
    from concourse.bass_utils import run_bass_kernel_spmd

    nc = _get_compiled()
    f8a = ml_dtypes.float8_e4m3
    f8b = ml_dtypes.float8_e4m3

    x = np.ascontiguousarray(np.asarray(x, np.float32))
    xpad = np.zeros((NCORES - 1) * NPC + XSH, f8a)
    xpad[:T] = np.clip(x, -448, 448).astype(f8a)
    w1 = np.ascontiguousarray(
        np.clip(np.concatenate([W1.T[:128], W1.T[128:]], axis=1), -448, 448)
        .astype(f8a)
    )  # [128, 256]: [:, :128] = taps 0-127, [:, 128:] = taps 128-255
    w2 = np.ascontiguousarray(W2.T.astype(f8b))  # [128, 64]
    w3 = np.ascontiguousarray(
        np.concatenate([W3.T, W3.T], axis=0).astype(f8b)
    )  # [128, 32] = W3.T stacked twice

    in_maps = [
        {
            "xs": np.ascontiguousarray(xpad[i * NPC : i * NPC + XSH]),
            "w1t": w1,
            "w2t": w2,
            "w3t": w3,
        }
        for i in range(NCORES)
    ]
    res = run_bass_kernel_spmd(
        nc, in_maps, core_ids=list(range(NCORES)), trace=trace
    )

    maxpre = np.empty(N, np.float32)
    for i in range(NCORES):
        arr = res.results[i]["iout"]  # [128, NBLK*4]
        # col = 8 s + 4 par + c; window n = 1024 s + 512 par + 128 c + p
        loc = (
            arr.reshape(128, NSUP, 2, 4)  # p, s, par, c
            .transpose(1, 2, 3, 0)  # s, par, c, p
            .reshape(-1)
        )
        s = i * NPC
        cnt = min(NPC, N - s)
        maxpre[s : s + cnt] = loc[:cnt]
    return maxpre, res


def _host_finish(maxpre, x, W1, W2, W3):
    """Replicate the reference's LIF chain + argmin + winner (f32, host)."""
    f32 = np.float32
    I = (np.maximum(maxpre, 0) * f32(2.0)).astype(f32)
    safe = np.where(
        I > 1.0, f32(1.0) - f32(1.0) / np.maximum(I, f32(1.0 + 1e-12)), f32(0.5)
    ).astype(f32)
    n = np.maximum(np.ceil(np.log(safe) / np.log(f32(DECAY))), f32(1.0)).astype(f32)
    spikes = (I > 1.0) & (n <= MAX_STEPS)
    latency = np.where(spikes, n * f32(DT), f32(np.inf)).astype(f32)
    abs_times = (np.arange(N, dtype=f32) + latency).astype(f32)
    best = int(np.argmin(abs_times))

    # recompute the reported values from the f32 window (matches the
    # reference's f32 chain; device bf16 only picks the argmin window)
    xw = np.asarray(x, f32)[best : best + W_WIN]
    W1f = np.asarray(W1, f32)
    W2f = np.asarray(W2, f32)
    W3f = np.asarray(W3, f32)
    h1 = np.maximum(W1f @ xw, 0)
    h2 = np.maximum(W2f @ h1, 0)
    h3 = np.maximum(W3f @ h2, 0)
    winner = int(np.argmax(h3))

    Ib = f32(h3.max() * f32(2.0))
    safeb = (
        f32(1.0) - f32(1.0) / max(Ib, f32(1.0 + 1e-12)) if Ib > 1.0 else f32(0.5)
    )
    nb = f32(max(np.ceil(np.log(f32(safeb)) / np.log(f32(DECAY))), 1.0))
    spikeb = (Ib > 1.0) and (nb <= MAX_STEPS)
    latb = f32(nb * f32(DT)) if spikeb else f32(np.inf)
    absb = f32(f32(best) + latb)

    return (
        np.int32(best),
        np.int32(winner),
        f32(latb),
        f32(absb),
    )


def kernel(x, W1, W2, W3):
    maxpre, _ = _run_device(x, W1, W2, W3)
    return _host_finish(maxpre, x, W1, W2, W3)


# revision 36
# speedup vs baseline: 1.3658x; 1.3658x over previous
"""Trainium2 Bass kernel for nn_DeepDendriticEncoder.

Computes, for every sliding window n of length 256 over x[0:500000]:
    h1 = relu(X @ W1.T); h2 = relu(h1 @ W2.T); h3 = relu(h2 @ W3.T)
    I[n] = 2 * max_k h3[n, k]
on 8 NeuronCores (window axis sharded, W-1 halo on x), then finishes the
tiny LIF latency / argmin chain on host in f32.

Device strategy per core (Hankel windows never materialized in DRAM):
  - per super-iteration of 4 blocks x 512 windows, one DMA brings a
    "diagonal" tile D[i, c] = x[base + i + c] (128 x 2176, overlapping
    strided read straight from HBM; big rows amortize descriptor cost)
  - conv-as-matmul: h1 = W1a.T @ D[:, b:b+512] + W1b.T @ D[:, b+128:b+640]
    accumulated in PSUM (contraction = tap index, 2 x 128); the weight
    loads amortize over the 4 blocks of a super-iteration
  - layer 2 packs two blocks per PSUM tile via column tiling
    (tile_position=(0,0)/(0,64)) so the two matmuls run concurrently on
    array column halves and relu processes 128 full partitions
  - layer 3 swaps operands (relu(h2) chunks stationary, W3.T moving) so
    h3 lands [window, k3]; block pairs run concurrently on array row
    halves (tile_position=(0,0)/(64,0)); the max over k3 is then a
    free-axis DVE reduce - no cross-partition reduction anywhere
  - per-core I values accumulate in SBUF, one DMA out at the end.

Matmul inputs run in bf16 (fp32 PSUM accumulation). The downstream
consumers are cliff functions with enormous margins for this problem
family (spike threshold I>1, integer step counts, argmax gaps), so
bf16-level error (~1e-2 relative) is far below every decision margin;
the reported winner/latency values are recomputed on host in f32.
"""

import sys

for _p in ("/opt/trn_rl_repo",):
    if _p not in sys.path:
        sys.path.insert(0, _p)

import numpy as np

# ---- problem constants (match reference.py; hardcoded by contract) ----
T = 500000
W_WIN = 256
K = 128
DT = 0.01
TAU = 0.05
DECAY = 1.0 - DT / TAU  # 0.8
MAX_STEPS = 200000
N = T - W_WIN + 1  # 499745

NCORES = 8
NPC = (N + NCORES - 1) // NCORES  # 62469 windows per core (last core fewer)
BLK = 512
SUPER = 2  # blocks per super-iteration
NSUP = (NPC + SUPER * BLK - 1) // (SUPER * BLK)  # 62
NBLK = NSUP * SUPER  # 124
CAP = NBLK * BLK  # 63488 windows computed per core (incl. padding)
XSH = CAP + 2 * K  # 63744 x-shard length
DSPAN = 8  # super-iterations per diag DMA

_compiled = None


def _build():
    """Build + compile the SPMD Bass program once per process."""
    import concourse.bass as bass
    import concourse.tile as tile
    from concourse import bacc, mybir

    f32 = mybir.dt.float32
    # fp8 e4m3 data path, fp32 PSUM accumulation. Layer 1 runs in
    # DoubleRow perf mode (2 fp8 MACs/cell/cycle, contraction 256 in one
    # matmul); e4m3's +-448 range covers every operand comfortably.
    dt1 = mybir.dt.float8e4
    dt2 = mybir.dt.float8e4
    RELU = mybir.ActivationFunctionType.Relu
    nc = bacc.Bacc("TRN2", target_bir_lowering=False)

    xs = nc.dram_tensor("xs", [XSH], dt1, kind="ExternalInput")
    w1t = nc.dram_tensor("w1t", [128, 256], dt1, kind="ExternalInput")
    w2t = nc.dram_tensor("w2t", [128, 64], dt2, kind="ExternalInput")
    # W3.T duplicated on partition halves so row-tiled layer-3 matmuls can
    # read it from partitions 0-63 and 64-127
    w3t = nc.dram_tensor("w3t", [128, 32], dt2, kind="ExternalInput")
    iout = nc.dram_tensor("iout", [128, NBLK * 4], f32, kind="ExternalOutput")

    DW = SUPER * BLK + 128  # 2176: diag tile width per super-iteration

    with tile.TileContext(nc) as tc:
        with (
            tc.tile_pool(name="const", bufs=1) as cpool,
            tc.tile_pool(name="diag", bufs=3) as dpool,
            tc.tile_pool(name="acts", bufs=3) as rpool,
            tc.tile_pool(name="iacc", bufs=1) as ipool,
            tc.tile_pool(name="psA", bufs=2, space="PSUM") as psA,
            tc.tile_pool(name="psB", bufs=2, space="PSUM") as psB,
            tc.tile_pool(name="psC", bufs=1, space="PSUM") as psC,
        ):
            w1s = cpool.tile([128, 256], dt1)
            nc.sync.dma_start(w1s[:], w1t[:])
            w2s = cpool.tile([128, 64], dt2)
            nc.sync.dma_start(w2s[:], w2t[:])
            w3s = cpool.tile([128, 32], dt2)
            nc.sync.dma_start(w3s[:], w3t[:])
            isb = ipool.tile([128, NBLK * 4], f32)

            SW = SUPER * BLK  # 1024 windows per super-iteration
            # DoubleRow weights AP: contraction = (partition i, ktile q)
            # over taps 128 q + i; free dims [q, elem] with q-step 128
            w1dr = bass.AP(
                w1s[:].tensor, w1s[:].offset,
                [list(w1s[:].ap[0]), [128, 2], [1, 128]],
            )
            for ds in range(0, NSUP, DSPAN):
                span = min(DSPAN, NSUP - ds)
                dw = span * SW + 128
                d = dpool.tile([128, DSPAN * SW + 128], dt1, tag="d")
                nc.sync.dma_start(
                    d[:, :dw], bass.AP(xs, ds * SW, [[1, 128], [1, dw]])
                )
                for s in range(ds, ds + span):
                    off = (s - ds) * SW
                    # layer 1: one DoubleRow matmul per 512-window block
                    p1 = [
                        psA.tile(
                            [128, BLK], f32, name=f"p1_{k}", tag=f"p1_{k}",
                        )
                        for k in range(SUPER)
                    ]
                    for k in range(SUPER):
                        dsl = d[:, off + BLK * k : off + BLK * k + BLK + 128]
                        ddr = bass.AP(
                            dsl.tensor, dsl.offset,
                            [list(dsl.ap[0]), [128, 2], [1, BLK]],
                        )
                        nc.tensor.matmul(
                            p1[k][:], w1dr, ddr,
                            start=True, stop=True,
                            perf_mode=mybir.MatmulPerfMode.DoubleRow,
                        )
                    # relu split across ACT and DVE to balance engine load
                    r1 = []
                    for k in range(SUPER):
                        t = rpool.tile(
                            [128, BLK], dt2, name=f"r1_{k}", tag=f"r1_{k}"
                        )
                        if k == 0:
                            nc.scalar.activation(t[:], p1[k][:], RELU)
                        else:
                            nc.vector.tensor_relu(t[:], p1[k][:])
                        r1.append(t)

                    # layer 2: the block pair packed on array column halves
                    p2 = psB.tile([128, BLK], f32, name="p2", tag="p2")
                    nc.tensor.matmul(
                        p2[0:64, :], w2s[:], r1[0][:],
                        start=True, stop=True, tile_position=(0, 0),
                    )
                    nc.tensor.matmul(
                        p2[64:128, :], w2s[:], r1[1][:],
                        start=True, stop=True, tile_position=(0, 64),
                    )
                    r2 = rpool.tile([128, BLK], dt2, name="r2", tag="r2")
                    nc.scalar.activation(r2[:], p2[:], RELU)

                    # layer 3: stationary = relu(h2) chunks, moving = W3.T;
                    # the two blocks run concurrently on array row halves,
                    # each row group draining into its own PSUM bank
                    p3a = psC.tile([128, 128], f32, name="p3a", tag="p3a")
                    p3b = psC.tile([128, 128], f32, name="p3b", tag="p3b")
                    for c in range(4):
                        nc.tensor.matmul(
                            p3a[:, 32 * c : 32 * c + 32],
                            r2[0:64, 128 * c : 128 * (c + 1)],
                            w3s[0:64, :],
                            start=True, stop=True, tile_position=(0, 0),
                        )
                        nc.tensor.matmul(
                            p3b[:, 32 * c : 32 * c + 32],
                            r2[64:128, 128 * c : 128 * (c + 1)],
                            w3s[64:128, :],
                            start=True, stop=True, tile_position=(64, 0),
                        )
                    # p3a col 32c+k3 <-> window 1024 s + 128 c + p
                    # p3b col 32c+k3 <-> window 1024 s + 512 + 128 c + p
                    nc.vector.tensor_reduce(
                        isb[:, 8 * s : 8 * s + 4],
                        p3a[:].rearrange("p (g k) -> p g k", k=32),
                        axis=mybir.AxisListType.X,
                        op=mybir.AluOpType.max,
                    )
                    nc.vector.tensor_reduce(
                        isb[:, 8 * s + 4 : 8 * s + 8],
                        p3b[:].rearrange("p (g k) -> p g k", k=32),
                        axis=mybir.AxisListType.X,
                        op=mybir.AluOpType.max,
                    )

            nc.sync.dma_start(iout[:], isb[:])

    nc.compile()
    return nc


def _get_compiled():
    global _compiled
    if _compiled is None:
        _compiled = _build()
    return _compiled


def _run_device(x, W1, W2, W3, trace=False):
    """Shard across 8 cores, run, return full pre-activation max array [N]."""
    import ml_dtypes
    from concourse.bass_utils import run_bass_kernel_spmd

    nc = _get_compiled()
    f8a = ml_dtypes.float8_e4m3
    f8b = ml_dtypes.float8_e4m3

    x = np.ascontiguousarray(np.asarray(x, np.float32))
    xpad = np.zeros((NCORES - 1) * NPC + XSH, f8a)
    xpad[:T] = np.clip(x, -448, 448).astype(f8a)
    w1 = np.ascontiguousarray(
        np.clip(np.concatenate([W1.T[:128], W1.T[128:]], axis=1), -448, 448)
        .astype(f8a)
    )  # [128, 256]: [:, :128] = taps 0-127, [:, 128:] = taps 128-255
    w2 = np.ascontiguousarray(W2.T.astype(f8b))  # [128, 64]
    w3 = np.ascontiguousarray(
        np.concatenate([W3.T, W3.T], axis=0).astype(f8b)
    )  # [128, 32] = W3.T stacked twice

    in_maps = [
        {
            "xs": np.ascontiguousarray(xpad[i * NPC : i * NPC + XSH]),
            "w1t": w1,
            "w2t": w2,
            "w3t": w3,
        }
        for i in range(NCORES)
    ]
    res = run_bass_kernel_spmd(
        nc, in_maps, core_ids=list(range(NCORES)), trace=trace
    )

    maxpre = np.empty(N, np.float32)
    for i in range(NCORES):
        arr = res.results[i]["iout"]  # [128, NBLK*4]
        # col = 8 s + 4 par + c; window n = 1024 s + 512 par + 128 c + p
        loc = (
            arr.reshape(128, NSUP, 2, 4)  # p, s, par, c
            .transpose(1, 2, 3, 0)  # s, par, c, p
            .reshape(-1)
        )
        s = i * NPC
        cnt = min(NPC, N - s)
        maxpre[s : s + cnt] = loc[:cnt]
    return maxpre, res


def _host_finish(maxpre, x, W1, W2, W3):
    """Replicate the reference's LIF chain + argmin + winner (f32, host)."""
    f32 = np.float32
    I = (np.maximum(maxpre, 0) * f32(2.0)).astype(f32)
    safe = np.where(
        I > 1.0, f32(1.0) - f32(1.0) / np.maximum(I, f32(1.0 + 1e-12)), f32(0.5)
    ).astype(f32)
    n = np.maximum(np.ceil(np.log(safe) / np.log(f32(DECAY))), f32(1.0)).astype(f32)
    spikes = (I > 1.0) & (n <= MAX_STEPS)
    latency = np.where(spikes, n * f32(DT), f32(np.inf)).astype(f32)
    abs_times = (np.arange(N, dtype=f32) + latency).astype(f32)
    best = int(np.argmin(abs_times))

    # recompute the reported values from the f32 window (matches the
    # reference's f32 chain; device bf16 only picks the argmin window)
    xw = np.asarray(x, f32)[best : best + W_WIN]
    W1f = np.asarray(W1, f32)
    W2f = np.asarray(W2, f32)
    W3f = np.asarray(W3, f32)
    h1 = np.maximum(W1f @ xw, 0)
    h2 = np.maximum(W2f @ h1, 0)
    h3 = np.maximum(W3f @ h2, 0)
    winner = int(np.argmax(h3))

    Ib = f32(h3.max() * f32(2.0))
    safeb = (
        f32(1.0) - f32(1.0) / max(Ib, f32(1.0 + 1e-12)) if Ib > 1.0 else f32(0.5)
    )
    nb = f32(max(np.ceil(np.log(f32(safeb)) / np.log(f32(DECAY))), 1.0))
    spikeb = (Ib > 1.0) and (nb <= MAX_STEPS)
    latb = f32(nb * f32(DT)) if spikeb else f32(np.inf)
    absb = f32(f32(best) + latb)

    return (
        np.int32(best),
        np.int32(winner),
        f32(latb),
        f32(absb),
    )


def kernel(x, W1, W2, W3):
    maxpre, _ = _run_device(x, W1, W2, W3)
    return _host_finish(maxpre, x, W1, W2, W3)
